# revision 21
# baseline (speedup 1.0000x reference)
"""GAT layer (nn_GATLayer_24249385353673) Trainium2 Bass kernel.

Sharding: data-parallel over batch b — core b computes batch element b.
No collectives. Each core:
  Wh = h_b @ W            [1024, 256]  (+ e1/e2 via extended weight matrix)
  w[j,i]  = lrelu(e1_i + e2_j) + mb[j,i]    (one fused custom DVE op/chunk;
                                             mb holds mask*(-1000) + SHIFT)
  P_T     = exp(w)                          (one wide ACT pass per head)
  outT[d,i] = sum_j Wh[j,d] * P_T[j,i]   (+ ones column -> denom row)
  out[i,d]  = outT[d,i] / denom[i]       (host transposes outT at gather)

Shapes hardcoded: B=8, N=1024, D_IN=256, D_OUT=256, H=8, HD=32, ALPHA=0.2.
"""

import os
from contextlib import ExitStack

import numpy as np

B, N, D_IN, D_OUT, H, HD = 8, 1024, 256, 256, 8, 32
ALPHA = 0.2
NEG_MASK = -1000.0  # additive post-lrelu mask value (exp -> exactly 0)
SHIFT = -4.0  # post-lrelu shift so exp() stays in fp16 range
N_CORES = 8
NC_CHUNKS = N // 128  # 8 node chunks of 128

USE_2X = True  # engage the hand-written 2x_1p uop program for the score op

_NC_CACHE = {}
LAST_RESULT = None  # BassKernelResults of the most recent run (for test.py)


def _register_score_op():
    """Fused DVE op: out = lrelu(in1 + s0) + in0
    (in1 = e1 broadcast tile, s0 = e2 per-partition column, in0 = mask bias
     with SHIFT folded in host-side, imm2 = leaky slope).

    Registered with a hand-built uop program: a REGULAR (1x) variant plus a
    2X_1PORT variant that computes two packed fp16 elements per cycle
    (perf_max=1).  Both variants are two states: a latch-init uop that loads
    the per-partition e2 scalar (C0) and the slope imm (C2) into per-stage
    swap flops, then the steady-state body."""
    import concourse.dve_ops as dve_ops_mod
    from concourse.dve_ops import DveOp, _COMPILE_CACHE
    from concourse.dve_spec import C0, C2, Spec, Src0, Src1, maxx
    from concourse.dve_table_gen import dve_ver_for
    from concourse.dve_uop import (
        AluInp,
        AluOp,
        DveOpSpec,
        InpSel,
        OutPath,
        OutSel,
        Trigger,
        UopConfig,
        UopDpConfig,
    )

    name = "GAT_SCORE2_ANT"
    if name in dve_ops_mod._SUB_OPCODE_FOR_NAME:
        return next(op for op in dve_ops_mod.OPS if op.name == name)

    _t = Src1 + C0
    spec = Spec(
        body=maxx(_t, _t * C2) + Src0,
        reference=lambda in0, in1, s0, s1, imm2: (
            np.maximum(in1.astype(np.float32) + s0, (in1.astype(np.float32) + s0) * imm2)
            + in0
        ).astype(np.float32),
    )

    PREV = AluInp.PREV_ALU_OUT
    SWAP = AluInp.CURR_SWAP_OUT
    L = [AluInp.PREV_DELAY_0, AluInp.PREV_DELAY_1, AluInp.PREV_DELAY_2,
         AluInp.PREV_DELAY_3, AluInp.PREV_DELAY_4, AluInp.PREV_DELAY_5]
    from concourse.dve_uop import DelayInp

    def _latch_init(swap_stages):
        """Latch-init uop: lane0 <- CONST_0 (e2 scalar), lane1 <- CONST_2
        (slope imm); write the swap flop at each (stage, lane_idx) in
        swap_stages via a BYPASS+swap stage."""
        u = UopConfig()
        u.enable_input(InpSel.CONST_0, 1)  # lane 0
        u.enable_input(InpSel.CONST_2, 2)  # lane 1
        u.trigger = (Trigger.COUNT, Trigger.NONE, Trigger.NONE)
        u.repeat_count = 1
        u.next_uop = (1, 0, 0)
        for st in range(8):
            blk = u.datapath_config[st]
            blk.pass_through_delay(0, 1)
            if st in swap_stages:
                src = L[swap_stages[st]]
                blk.enable_alu(AluOp.BYPASS, src, src)
                blk.swap_enable = 1
        # no stream consume, no write
        return u

    def _steady_1x():
        u = UopConfig()
        # slot0: SRC_1 (e1, read at stage0 via mux sel PREV_ALU_OUT)
        # slot1 -> lane0: SRC_0 (mb, read at stage 3)
        u.enable_input(InpSel.SRC_1, 0)
        u.enable_input(InpSel.SRC_0, 1)
        u.require_inp0 = 1
        u.require_inp1 = 1
        u.trigger = (Trigger.SRC_TENSOR_DONE, Trigger.NONE, Trigger.NONE)
        u.next_uop = (0, 0, 0)
        dp = u.datapath_config
        for st in range(8):
            dp[st].pass_through_delay(0)
        # s0: t = e1 + e2(swap)
        dp[0].enable_alu(AluOp.ADD, PREV, SWAP)
        # s1: u = t * alpha(swap); lane1 <- t
        dp[1].enable_alu(AluOp.MULTIPLY, PREV, SWAP)
        dp[1].enable_delay_from_src(DelayInp.PREV_ALU_OUT, 1)
        # s2: m = max(u, t)
        dp[2].enable_alu(AluOp.MAX, PREV, L[1])
        # s3: o = m + mb
        dp[3].enable_alu(AluOp.ADD, PREV, L[0])
        # s4-7: carry o through the ALU chain
        for st in range(4, 8):
            dp[st].pass_through_alu()
        u.enable_output(OutSel.ALU_OUT, OutPath.WR0_LO)
        return u

    def _steady_2x():
        # Mirrors stock TT 2X_1PORT exactly in its I/O wiring: input slots
        # 0-3 carry SRC_0 / SRC_1 / SRC_0_HI / SRC_1_HI (the _HI extractors
        # appear to be hardwired to slots 2/3), the write port reads the LO
        # result from ALU_OUT (last block) and the HI result from DELAY_0.
        # So: HI element computed in stages 0-3, parked on lane 0 (reclaimed
        # read-before-write at s4); LO element in stages 4-7 ends at ALU_OUT.
        u = UopConfig()
        # slot0 = SRC_0 (mb lo; block0 copies it to lane5 for the s7 read)
        # slot1 -> lane0: SRC_1 (e1 lo; read at s4, lane then holds o_hi)
        # slot2 -> lane1: SRC_0_HI (mb hi; read at s3)
        # slot3 -> lane2: SRC_1_HI (e1 hi; read at s0)
        u.enable_input(InpSel.SRC_0, 0)
        u.enable_input(InpSel.SRC_1, 1)
        u.enable_input(InpSel.SRC_0_HI, 2)
        u.enable_input(InpSel.SRC_1_HI, 3)
        u.require_inp0 = 1
        u.require_inp1 = 1
        u.trigger = (Trigger.SRC_TENSOR_DONE, Trigger.NONE, Trigger.NONE)
        u.next_uop = (0, 0, 0)
        dp = u.datapath_config
        for st in range(8):
            dp[st].pass_through_delay(0, 1, 2, 5)
        # s0: t_hi = e1_hi + e2(swap); lane5 <- slot0 (mb lo)
        dp[0].enable_alu(AluOp.ADD, L[2], SWAP)
        dp[0].enable_delay_from_src(DelayInp.PREV_ALU_OUT, 5)
        # s1: u_hi = t_hi * alpha(swap); lane3 <- t_hi
        dp[1].enable_alu(AluOp.MULTIPLY, PREV, SWAP)
        dp[1].enable_delay_from_src(DelayInp.PREV_ALU_OUT, 3)
        dp[2].pass_through_delay(3)
        # s2: m_hi = max(u_hi, t_hi)
        dp[2].enable_alu(AluOp.MAX, PREV, L[3])
        # s3: o_hi = m_hi + mb_hi
        dp[3].enable_alu(AluOp.ADD, PREV, L[1])
        # s4: t_lo = e1_lo + e2(swap); lane0 <- o_hi (reads e1_lo first)
        dp[4].enable_alu(AluOp.ADD, L[0], SWAP)
        dp[4].enable_delay_from_src(DelayInp.PREV_ALU_OUT, 0)
        # s5: u_lo = t_lo * alpha(swap); lane3 <- t_lo
        dp[5].enable_alu(AluOp.MULTIPLY, PREV, SWAP)
        dp[5].enable_delay_from_src(DelayInp.PREV_ALU_OUT, 3)
        dp[6].pass_through_delay(3)
        # s6: m_lo = max(u_lo, t_lo)
        dp[6].enable_alu(AluOp.MAX, PREV, L[3])
        # s7: o_lo = m_lo + mb_lo
        dp[7].enable_alu(AluOp.ADD, PREV, L[5])
        u.enable_output(OutSel.ALU_OUT, OutPath.WR0_LO)  # o_lo from stage 7
        u.enable_output(OutSel.DELAY_0, OutPath.WR0_HI)  # o_hi rides lane 0
        return u

    # swap flops: 1x uses stages {0: e2, 1: alpha}; 2x adds {4: e2, 5: alpha}
    # NOTE (HW-measured): with perf_max=1 the engine packs the I/O into
    # 2-elements/cycle mode but still executes the program at table_ptr+0.
    # So the REGULAR slot carries the two-element program when USE_2X.
    if USE_2X:
        uops_1x = [_latch_init({0: 0, 1: 1, 4: 0, 5: 1}), _steady_2x()]
    else:
        uops_1x = [_latch_init({0: 0, 1: 1}), _steady_1x()]
    uops_2x = [_latch_init({0: 0, 1: 1, 4: 0, 5: 1}), _steady_2x()]

    row = max(dve_ops_mod._SUB_OPCODE_FOR_NAME.values(), default=0) + 1
    assert row < 0x20
    op = DveOp(name, spec, subdim=False, uops_sha={})
    dve_ops_mod.OPS.append(op)
    dve_ops_mod._SUB_OPCODE_FOR_NAME[name] = row
    dve_ops_mod.CUSTOM_DVE_SPECS[name] = spec
    for trn in ("TRN2",):
        ver = dve_ver_for(trn)
        compiled = DveOpSpec(
            name=name,
            opcode=row,
            uops=uops_1x,
            uops_2x=None,
            perf_max=1 if USE_2X else 0,
            rd1_en=True,
        )
        compiled.validate(ver)
        _COMPILE_CACHE[(name, ver)] = compiled
        op.uops_sha[ver] = compiled.sha(ver)
    return op


def _patch_tile_drain():
    """This container's walrus build only encodes ONE sync wait per
    instruction; Tile's kernel-tail drain carries one wait per live
    semaphore. Split the waits across follow-up sync-engine nops."""
    import concourse.tile as tile
    from concourse.vector_clock import ScopedClock

    if getattr(tile.TileContext, "_gat_drain_patched", False):
        return

    def _drain_and_barrier(self, tick_clock, wait_clock):
        nc = self.nc
        drain_inst = nc.sync.drain()
        wait_clock.add_sem_waits(
            drain_inst.ins, ScopedClock({None: tick_clock.global_clock})
        )
        si = drain_inst.ins.sync_info
        waits = list(si.on_wait)
        if len(waits) > 1:
            si.on_wait = waits[:1]
            drain_inst.ins.sync_info = si
            si_cls = type(si)
            for w in waits[1:]:
                nop = nc.sync.nop()
                nop.ins.sync_info = si_cls(on_wait=[w], on_update=[])
        nc.all_engine_barrier()
        assert self.sems is not None
        popped = nc._tile_sem_poison_stack.pop()
        assert popped is self._sem_poison
        nc.clear_and_free_semaphores(list(self.sems.allocated().values()))
        nc.all_engine_barrier()

    tile.TileContext._drain_and_barrier = _drain_and_barrier
    tile.TileContext._gat_drain_patched = True


def _split_multi_waits(nc):
    """This walrus build encodes at most ONE sync wait per instruction.
    Move excess waits onto same-engine NoOps inserted just before the
    offending instruction (engines execute their stream in order, so
    hoisting waits to earlier slots on the same engine is equivalent)."""
    import concourse.mybir as mybir

    si_cls = None
    n_new = 0
    for f in nc.m.functions:
        for bb in f.blocks:
            insts = bb.instructions
            out = []
            for inst in insts:
                si = inst.sync_info
                waits = list(si.on_wait) if si is not None else []
                if len(waits) > 1:
                    if si_cls is None:
                        si_cls = type(si)
                    for w in waits[:-1]:
                        nop = mybir.InstNoOp(
                            name=f"waitnop-{n_new}",
                            ins=[],
                            outs=[],
                            engine=inst.engine,
                        )
                        nop.sync_info = si_cls(on_wait=[w], on_update=[])
                        out.append(nop)
                        n_new += 1
                    si.on_wait = waits[-1:]
                    inst.sync_info = si
                out.append(inst)
            if n_new:
                insts[:] = out
    return n_new


def _build_nc(split_waits=True):
    import concourse.bass as bass
    import concourse.mybir as mybir
    import concourse.tile as tile
    from concourse.masks import make_identity

    _patch_tile_drain()
    score_op = _register_score_op()

    f32 = mybir.dt.float32
    f16 = mybir.dt.float16
    AF = mybir.ActivationFunctionType

    nc = bass.Bass()
    htb_d = nc.dram_tensor("htb", [D_IN, N], mybir.dt.bfloat16, kind="ExternalInput")
    wexb_d = nc.dram_tensor(
        "wexb", [D_IN, D_OUT + 2 * H], mybir.dt.bfloat16, kind="ExternalInput"
    )
    mb_d = nc.dram_tensor("mb", [N, N], f16, kind="ExternalInput")
    outT_d = nc.dram_tensor("outT", [D_OUT, N], f32, kind="ExternalOutput")
    e1_scratch = nc.dram_tensor("e1_scratch", [1, H * N], f16)
    dn_d = nc.dram_tensor("dn", [H, N], f32, kind="ExternalOutput")

    with tile.TileContext(nc) as tc, ExitStack() as ctx:
        const = ctx.enter_context(tc.tile_pool(name="const", bufs=1))
        ident = const.tile([128, 128], f32)
        make_identity(nc, ident[:])

        wex_pool = ctx.enter_context(tc.tile_pool(name="wex", bufs=1))
        mb_pool = ctx.enter_context(tc.tile_pool(name="mb", bufs=1))
        ht_pool = ctx.enter_context(tc.tile_pool(name="ht", bufs=2))
        wh_pool = ctx.enter_context(tc.tile_pool(name="wh", bufs=NC_CHUNKS))
        e_pool = ctx.enter_context(tc.tile_pool(name="e", bufs=NC_CHUNKS))
        e1r_pool = ctx.enter_context(tc.tile_pool(name="e1r", bufs=1))
        e1b_pool = ctx.enter_context(tc.tile_pool(name="e1b", bufs=1))
        w_pool = ctx.enter_context(tc.tile_pool(name="w", bufs=3))
        p_pool = ctx.enter_context(tc.tile_pool(name="p", bufs=5))
        stage_pool = ctx.enter_context(tc.tile_pool(name="stage", bufs=2))

        # ---- DMA inputs in.  wexb/htb first (they gate the e1 chain, the
        # critical path to the first score op); mask chunks ride the GPSIMD
        # queue so they never block the sync queue's e1 scratch/broadcasts.
        bf16 = mybir.dt.bfloat16
        wexb_all = wex_pool.tile([128, 2, D_OUT + 2 * H], bf16, tag="wexb")
        nc.sync.dma_start(
            wexb_all[:], wexb_d[:].rearrange("(k p) d -> p k d", p=128)
        )
        wexb_sb = [wexb_all[:, kc, :] for kc in range(2)]
        htb_all = ht_pool.tile([128, 2, N], bf16, tag="htb")
        nc.sync.dma_start(
            htb_all[:], htb_d[:].rearrange("(k p) d -> p k d", p=128)
        )
        htb_sb = [htb_all[:, kc, :] for kc in range(2)]
        # warm the exp activation table first thing on the scalar engine
        warm = const.tile([1, 8], f32, tag="warm")
        nc.vector.memset(warm[:], 0.0)
        nc.scalar.activation(warm[:], warm[:], AF.Exp)
        # tiny scalar-engine read of htb: the mask DMA triggers emitted after
        # this only fire once htb has fully landed, keeping the 2MB mask
        # transfer off the critical htb -> e-matmul -> e1-broadcast path
        htb_gate = const.tile([1, 8], bf16, tag="htb_gate")
        nc.scalar.copy(htb_gate[:], htb_all[0:1, 0, 0:8])
        mb_all = mb_pool.tile([128, NC_CHUNKS, N], f16, tag="mb")
        for c in range(NC_CHUNKS):
            nc.scalar.dma_start(
                mb_all[:, c, :], mb_d[c * 128 : (c + 1) * 128, :]
            )
        mb_sb = [mb_all[:, c, :] for c in range(NC_CHUNKS)]

        # ---- matmul1: e1/e2 columns FIRST (tiny, unblocks the e1 row
        # broadcast chain), then the full Wh ----
        wh_sb = []  # [128, H, HD+1] fp16 per node chunk: [Wh_head | ones]
        e_sb = []  # [128, 16] f32 per node chunk: cols 0:8 e1, 8:16 e2
        with tc.tile_pool(name="psum_e", bufs=2, space="PSUM") as psE, tc.tile_pool(
            name="psum_mm1", bufs=2, space="PSUM"
        ) as psB, tc.tile_pool(name="psum_e1t", bufs=1, space="PSUM") as psT:
            for c in range(NC_CHUNKS):
                pe_ = psE.tile([128, 2 * H], f32, tag="mme", name=f"mme{c}")
                for kc in range(2):
                    nc.tensor.matmul(
                        pe_[:],
                        htb_sb[kc][:, c * 128 : (c + 1) * 128],
                        wexb_sb[kc][:, D_OUT : D_OUT + 2 * H],
                        start=(kc == 0),
                        stop=(kc == 1),
                    )
                et = e_pool.tile([128, 2 * H], f32, tag="e")
                nc.vector.tensor_copy(et[:], pe_[:])
                e_sb.append(et)
            # e1 rows: transpose e1 columns -> [8, 1024], then fp16
            e1t = psT.tile([8, N], f32, tag="e1t")
            for c in range(NC_CHUNKS):
                nc.tensor.transpose(
                    e1t[:, c * 128 : (c + 1) * 128], e_sb[c][:, 0:H], ident[:]
                )
            e1r = e1r_pool.tile([8, N], f16, tag="e1r")
            nc.vector.tensor_copy(e1r[:], e1t[:])
            nc.sync.dma_start(e1_scratch[:], e1r[:])
            for c in range(NC_CHUNKS):
                p1 = psB.tile([128, D_OUT], f32, tag="mm1")
                for kc in range(2):
                    nc.tensor.matmul(
                        p1[:],
                        htb_sb[kc][:, c * 128 : (c + 1) * 128],
                        wexb_sb[kc][:, 0:D_OUT],
                        start=(kc == 0),
                        stop=(kc == 1),
                    )
                wt = wh_pool.tile([128, H, HD + 1], f16, tag="wh")
                nc.vector.tensor_copy(
                    wt[:, :, 0:HD], p1[:].rearrange("p (h q) -> p h q", h=H)
                )
                nc.vector.memset(wt[:, :, HD : HD + 1], 1.0)
                wh_sb.append(wt)

        # e1 broadcast: zero-stride DRAM reads replicate each head's row
        # across all 128 partitions.  One TILE per head so a head's first
        # score op only waits for its own broadcast, not all eight.
        e1b_sb = []
        for hh in range(H):
            e1b_t = e1b_pool.tile([128, N], f16, tag=f"e1b{hh}", name=f"e1b{hh}")
            nc.scalar.dma_start(
                e1b_t[:],
                e1_scratch[0:1, hh * N : (hh + 1) * N].partition_broadcast(128),
            )
            e1b_sb.append(e1b_t)

        # ---- main loop: fused scores (DVE) -> wide exp (ACT) -> matmuls.
        # Heads are processed in PAIRS; each pair's attention matmuls run as
        # two concurrent column-group tiles of the PE array (h0 at array cols
        # 0-32, h1 at 64-96), so two 512-col streams cost ~one stream time.
        with tc.tile_pool(name="psum_mm2", bufs=6, space="PSUM") as ps2:
            for pair in range(H // 2):
                h0, h1 = 2 * pair, 2 * pair + 1
                p_alls = {}
                for hh in (h0, h1):
                    e1b = e1b_sb[hh]
                    wt_all = w_pool.tile(
                        [128, NC_CHUNKS, N], f16, tag="wsc", name=f"wsc{hh}"
                    )
                    for jc in range(NC_CHUNKS):
                        e2col = e_sb[jc][:, H + hh : H + hh + 1]
                        inst = nc.vector._custom_dve(
                            score_op,
                            out=wt_all[:, jc, :],
                            in0=mb_sb[jc][:],
                            in1=e1b[:],
                            s0=e2col,
                            s1=0.0,
                            imm2=ALPHA,
                        )
                        if USE_2X:
                            inst.ins.perf_max = 1
                    # one wide exp pass for the whole head
                    p_all = p_pool.tile(
                        [128, NC_CHUNKS, N], f16, tag="p", name=f"p{hh}"
                    )
                    nc.scalar.activation(
                        p_all[:].rearrange("p c n -> p (c n)"),
                        wt_all[:].rearrange("p c n -> p (c n)"),
                        AF.Exp,
                    )
                    p_alls[hh] = p_all
                # attention matmuls: accumulate over j chunks; h0/h1 go to
                # column groups 0 / 64 of the PE array and run concurrently
                acc = [
                    ps2.tile([128, 512], f32, tag="mm2", name=f"acc{pair}_{i}")
                    for i in range(2)
                ]
                for jc in range(NC_CHUNKS):
                    for ic in range(2):
                        for base, hh in ((0, h0), (64, h1)):
                            nc.tensor.matmul(
                                acc[ic][base : base + HD + 1, :],
                                wh_sb[jc][:, hh, :],
                                p_alls[hh][:, jc, ic * 512 : (ic + 1) * 512],
                                start=(jc == 0),
                                stop=(jc == NC_CHUNKS - 1),
                            )
                # evacuate PSUM -> SBUF staging (one full-width copy per
                # half), then DMA the unnormalized numerators and denominator
                # rows straight to HBM; the host divides during the gather.
                g = pair // 2
                stage = stage_pool.tile(
                    [128, 2, 512], f32, tag="stage", name=f"st{pair}"
                )
                nc.vector.tensor_copy(stage[:, 0, :], acc[0][:])
                nc.vector.tensor_copy(stage[:, 1, :], acc[1][:])
                for ic in range(2):
                    for base, hh in ((0, h0), (64, h1)):
                        k = hh % 4
                        nc.sync.dma_start(
                            outT_d[
                                g * 128 + k * HD : g * 128 + (k + 1) * HD,
                                ic * 512 : (ic + 1) * 512,
                            ],
                            stage[base : base + HD, ic, :],
                        )
                        nc.scalar.dma_start(
                            dn_d[hh : hh + 1, ic * 512 : (ic + 1) * 512],
                            stage[base + HD : base + HD + 1, ic, :],
                        )

    from concourse.library_overlay import lower_extended_insts

    lower_extended_insts(nc)  # populate .instr bytes for InstCustomDveAnt
    if split_waits:
        _split_multi_waits(nc)
    return nc


def _get_nc():
    if "nc" not in _NC_CACHE:
        _NC_CACHE["nc"] = _build_nc()
    return _NC_CACHE["nc"]


def _prep_inputs(h, adj_mask, W, a):
    hT = np.ascontiguousarray(np.swapaxes(np.asarray(h, dtype=np.float32), 1, 2))
    adj = np.asarray(adj_mask)
    W = np.asarray(W, dtype=np.float32)
    a = np.asarray(a, dtype=np.float32)

    # maskbias, transposed, with the exp-shift folded in:
    # mb[b, j, i] = SHIFT + (0 if adj[b, i, j] else NEG_MASK)   (post-lrelu add)
    mb = np.where(
        np.swapaxes(adj, 1, 2) == 0,
        np.float16(NEG_MASK + SHIFT),
        np.float16(SHIFT),
    ).astype(np.float16)

    Wr = W.reshape(D_IN, H, HD)
    w1 = Wr @ a[:HD]  # [D_IN, H]
    w2 = Wr @ a[HD:]  # [D_IN, H]
    wex = np.ascontiguousarray(
        np.concatenate([W, w1, w2], axis=1), dtype=np.float32
    )
    import ml_dtypes
    htb = hT.astype(ml_dtypes.bfloat16)
    wexb = wex.astype(ml_dtypes.bfloat16)
    return mb, htb, wexb


def kernel(h, adj_mask, W, a):
    global LAST_RESULT
    # persistent jax/XLA cache: repeat calls (and reruns) skip the multi-
    # minute neuronx-cc compile for an unchanged module
    os.environ.setdefault("JAX_COMPILATION_CACHE_DIR", "/tmp/jax_bass_cache")
    from concourse.bass_utils import run_bass_kernel_spmd

    mb_np, htb_np, wexb_np = _prep_inputs(h, adj_mask, W, a)
    nc = _get_nc()

    core_ids = list(range(N_CORES))
    in_maps = [
        {
            "htb": np.ascontiguousarray(htb_np[b]),
            "mb": np.ascontiguousarray(mb_np[b]),
            "wexb": wexb_np,
        }
        for b in range(N_CORES)
    ]
    res = run_bass_kernel_spmd(nc, in_maps, core_ids)
    LAST_RESULT = res
    out = np.empty((N_CORES, N, D_OUT), dtype=np.float32)
    for b in range(N_CORES):
        num = np.asarray(res.results[b]["outT"], dtype=np.float32)  # [256, N]
        den = np.asarray(res.results[b]["dn"], dtype=np.float32)  # [H, N]
        num = num.reshape(2, 4, HD, N)
        for hh in range(H):
            gg, kk = hh // 4, hh % 4
            out[b, :, hh * HD : (hh + 1) * HD] = (
                num[gg, kk] / den[hh][None, :]
            ).T
    return out
